# revision 1
# baseline (speedup 1.0000x reference)
"""Trainium2 Bass kernel for nn_Block_19524921327813 (moe_routing).

Mixture-of-depths block: router top-k (CAP=1024 of S=2048) -> gathered
q path (qln, q-proj, rope) + full-seq kv path (vln, kv-proj, rope) ->
MHA -> out-proj + SwiGLU FFN -> weighted scatter-add into seq.

Sharding: 8 cores = 4 batches x 2 query-halves. Each core runs the full
kv pipeline for its batch (duplicated within the pair) and 512 of the
1024 selected query tokens. No cross-core communication; the host
slices inputs and scatter-adds the per-core updates into the output.

All matmuls run as float32r (~1.4e-4 rel err at full PE rate).
"""
import numpy as np

import concourse.bacc as bacc
import concourse.mybir as mybir
import concourse.tile as tile
from concourse.bass_utils import run_bass_kernel_spmd
from concourse.alu_op_type import AluOpType

F32 = mybir.dt.float32
F32R = mybir.dt.float32r
I16 = mybir.dt.int16
U8 = mybir.dt.uint8
U32 = mybir.dt.uint32
AF = mybir.ActivationFunctionType
OP = AluOpType
AX = mybir.AxisListType

B, S, D, H, HD = 4, 2048, 1024, 16, 64
CAP = S // 2          # 1024 selected tokens per batch
NQ = CAP // 2         # 512 query tokens per core
ROPE_BASE = 10000.0
LN_EPS = 1e-5
NCH = D // 128        # 8 d-chunks
NSCH = S // 128       # 16 s-tiles
NJCH = 8              # 2-head blocks
VE = HD + 2           # v-ext cols per head: 64 v + 2 ones (even M)


def build_program(num_devices):
    nc = bacc.Bacc("TRN2", target_bir_lowering=False, debug=False,
                   num_devices=num_devices)

    def din(name, shape, dtype=F32):
        return nc.dram_tensor(name, shape, dtype, kind="ExternalInput").ap()

    seqT_d = din("seqT", [D, S])
    seqn_d = din("seqn", [S, D])
    rowsel_d = din("rowsel", [128, 1], I16)
    statw_d = din("statw", [D, 2])
    kvwk_d = din("kvwk", [D, D])
    kvwv_d = din("kvwv", [D, D])
    qw_d = din("qw", [D, D])
    ow_d = din("ow", [D, D])
    f1w_d = din("f1w", [D, 2 * D])
    f2w_d = din("f2w", [D, D])
    rotk_d = din("rotk", [128, S])
    rotq_d = din("rotq", [S, 128])
    ident_d = din("ident", [128, 128])
    tile16_d = din("tile16", [16, 128])
    iota1_d = din("iota1", [16, 128])
    vg_d = din("vg", [1, D])
    vbg_d = din("vbg", [2, D])
    qg_d = din("qg", [128, NCH])
    qb_d = din("qb", [128, NCH])
    f1b_d = din("f1b", [128, 16])
    f2b_d = din("f2b", [128, NCH])

    upd_d = nc.dram_tensor("updT", [128, NCH, NQ], F32,
                           kind="ExternalOutput").ap()
    idx_d = nc.dram_tensor("idxsel", [16, 32], F32, kind="ExternalOutput").ap()

    tc_cm = tile.TileContext(nc)
    tc = tc_cm.__enter__()
    try:
        _emit(nc, tc, locals())
    finally:
        tc_cm.__exit__(None, None, None)
    nc.compile()
    return nc


def _emit(nc, tc, d):
    seqT_d, seqn_d, rowsel_d, statw_d = d["seqT_d"], d["seqn_d"], d["rowsel_d"], d["statw_d"]
    kvwk_d, kvwv_d, qw_d, ow_d = d["kvwk_d"], d["kvwv_d"], d["qw_d"], d["ow_d"]
    f1w_d, f2w_d, rotk_d, rotq_d = d["f1w_d"], d["f2w_d"], d["rotk_d"], d["rotq_d"]
    ident_d, tile16_d, iota1_d = d["ident_d"], d["tile16_d"], d["iota1_d"]
    vg_d, vbg_d, qg_d, qb_d = d["vg_d"], d["vbg_d"], d["qg_d"], d["qb_d"]
    f1b_d, f2b_d, upd_d, idx_d = d["f1b_d"], d["f2b_d"], d["upd_d"], d["idx_d"]
    from contextlib import ExitStack

    with ExitStack() as gctx:
        pc = gctx.enter_context(tc.tile_pool(name="const", bufs=1))
        rt = gctx.enter_context(tc.tile_pool(name="route", bufs=2))
        dram = gctx.enter_context(tc.tile_pool(name="dram", bufs=1, space="DRAM"))

        ident = pc.tile([128, 128], F32)
        nc.sync.dma_start(ident[:], ident_d)
        tile16 = pc.tile([16, 128], F32R)
        nc.sync.dma_start(tile16[:], tile16_d.bitcast(F32R))
        iota1 = pc.tile([16, 128], F32)
        nc.sync.dma_start(iota1[:], iota1_d)
        ones1f = pc.tile([1, 128], F32)
        nc.vector.memset(ones1f[:], 1.0)
        ones1 = pc.tile([1, 128], F32R)
        nc.vector.tensor_copy(ones1[:], ones1f[:])
        vg = pc.tile([1, D], F32R)
        nc.sync.dma_start(vg[:], vg_d.bitcast(F32R))
        vbg = pc.tile([2, D], F32R)
        nc.sync.dma_start(vbg[:], vbg_d.bitcast(F32R))
        qg = pc.tile([128, NCH], F32)
        nc.sync.dma_start(qg[:], qg_d)
        qb = pc.tile([128, NCH], F32)
        nc.sync.dma_start(qb[:], qb_d)
        f1b = pc.tile([128, 16], F32)
        nc.sync.dma_start(f1b[:], f1b_d)
        f2b = pc.tile([128, NCH], F32)
        nc.sync.dma_start(f2b[:], f2b_d)
        rowsel = pc.tile([128, 1], I16)
        nc.sync.dma_start(rowsel[:], rowsel_d)
        statw = pc.tile([128, NCH, 2], F32R)
        nc.sync.dma_start(statw[:],
                          statw_d.rearrange("(c p) m -> p c m", p=128).bitcast(F32R))
        idx16 = pc.tile([128, 32], I16)

        scr_l = dram.tile([S], F32)
        scr_c = dram.tile([CAP], F32)
        scr_m = dram.tile([NQ], F32)
        qspill = dram.tile([128, NCH, NQ], F32R)

        with ExitStack() as vctx:
            pvln = vctx.enter_context(tc.tile_pool(name="vlnTp", bufs=1))
            vlnT = pvln.tile([128, NCH, S], F32R)
            rowsB = vctx.enter_context(tc.tile_pool(name="rowsB", bufs=1))
            l_row = rowsB.tile([1, S], F32)
            e_row = rowsB.tile([1, S], F32R)
            maxl = rowsB.tile([1, 1], F32)
            nmaxl = rowsB.tile([1, 1], F32)
            zr = rowsB.tile([1, 1], F32)
            rzr = rowsB.tile([1, 1], F32)

            # ============ A1: stats + routing + vlnT ============
            with tc.tile_pool(name="seqs", bufs=1) as pseq, \
                 tc.tile_pool(name="rowsA", bufs=1) as rowsA, \
                 tc.tile_pool(name="sqp", bufs=1) as psq, \
                 tc.tile_pool(name="rt_ps", bufs=1, space="PSUM") as rt_ps, \
                 tc.tile_pool(name="ps_st", bufs=1, space="PSUM") as ps_st, \
                 tc.tile_pool(name="ps_bc", bufs=1, space="PSUM") as ps_bc:

                seqT = pseq.tile([128, NCH, S], F32R)
                nc.sync.dma_start(
                    seqT[:], seqT_d.rearrange("(c p) s -> p c s", p=128).bitcast(F32R))
                st1 = ps_st.tile([2, S], F32, tag="st")
                for nb in range(4):
                    sl = slice(512 * nb, 512 * (nb + 1))
                    for ch in range(NCH):
                        nc.tensor.matmul(st1[:, sl], statw[:, ch, :],
                                         seqT[:, ch, sl],
                                         start=(ch == 0), stop=(ch == NCH - 1))
                mu_t = rowsA.tile([2, S], F32)
                nc.vector.tensor_scalar(mu_t[:, :], st1[:, :], 1.0 / D, None,
                                        op0=OP.mult)
                nc.vector.tensor_copy(l_row[:], st1[0:1, :])
                nc.vector.reduce_max(maxl[:], l_row[:], axis=AX.X)
                nc.vector.tensor_scalar(nmaxl[:], maxl[:], -1.0, None, op0=OP.mult)
                nc.scalar.activation(e_row[:], l_row[:], AF.Exp, bias=nmaxl[:])
                nc.vector.reduce_sum(zr[:], e_row[:].bitcast(F32), axis=AX.X)
                nc.vector.reciprocal(rzr[:], zr[:])
                nc.sync.dma_start(scr_l[:], l_row[:])

                st2 = ps_st.tile([2, S], F32, tag="st")
                for ch in range(NCH):
                    for nb in range(4):
                        sl = slice(512 * nb, 512 * (nb + 1))
                        sq = psq.tile([128, 512], F32R, tag="sq")
                        nc.scalar.activation(sq[:], seqT[:, ch, sl].bitcast(F32),
                                             AF.Square)
                        nc.tensor.matmul(st2[:, sl], statw[:, ch, :], sq[:],
                                         start=(ch == 0), stop=(ch == NCH - 1))
                var_t = rowsA.tile([2, S], F32)
                nc.vector.tensor_tensor(var_t[:, :], mu_t[:, :], mu_t[:, :],
                                        op=OP.mult)
                nc.vector.scalar_tensor_tensor(var_t[:, :], st2[:, :], 1.0 / D,
                                               var_t[:, :], op0=OP.mult,
                                               op1=OP.subtract)
                sd_t = rowsA.tile([2, S], F32)
                eps2 = rowsA.tile([2, 1], F32)
                nc.vector.memset(eps2[:], LN_EPS)
                nc.vector.tensor_scalar(var_t[:, :], var_t[:, :], 0.0, None,
                                        op0=OP.max)
                nc.scalar.activation(sd_t[:, :], var_t[:, :], AF.Sqrt,
                                     bias=eps2[:, :])
                with nc.allow_low_precision(reason="f32r rounding intended"):
                    nc.vector.reciprocal(sd_t[:, :].bitcast(F32R), sd_t[:, :])
                rstd_t = sd_t.bitcast(F32R) if False else sd_t
                # reuse var_t as -mu*rstd scratch, then shift rows to partition 0
                nc.vector.scalar_tensor_tensor(var_t[:, :], mu_t[:, :], -1.0,
                                               rstd_t[:, :],
                                               op0=OP.mult, op1=OP.mult)
                rstd0 = rowsA.tile([1, S], F32R)
                nc.sync.dma_start(rstd0[:], rstd_t[1:2, :].bitcast(F32R))
                nmr = rowsA.tile([2, S], F32R)
                nc.vector.tensor_scalar(nmr[0:1, :], e_row[:].bitcast(F32), 0.0,
                                        1.0, op0=OP.mult, op1=OP.add)
                nc.sync.dma_start(nmr[1:2, :], var_t[1:2, :].bitcast(F32R))

                # routing chain (gpsimd)
                l128 = rt.tile([128, 16], F32)
                nc.sync.dma_start(l128[:], scr_l[:].rearrange("(f p) -> p f", p=128))
                l16 = rt.tile([16, 128], F32)
                nc.sync.dma_start(l16[:], scr_l[:].rearrange("(f p) -> p f", p=16))
                neg = rt.tile([128, 16], F32, tag="neg")
                nc.vector.memset(neg[:], -1e30)
                cur = l128
                nv = [2048, 1538, 1028]
                ks = [509, 509, 3]
                kout = None
                for i in range(3):
                    kout = rt.tile([1, 2], F32, tag="kout")
                    nc.gpsimd.kth_largest(kout[:], cur[:], n_per_lane=16, k=510,
                                          quantile=1.0 - (ks[i] + 0.5) / (nv[i] - 1.0))
                    if i < 2:
                        tb = rt.tile([128, 2], F32, tag="tb")
                        nc.gpsimd.partition_broadcast(tb[:], kout[:])
                        gt = rt.tile([128, 16], U8, tag="gt")
                        nc.vector.tensor_scalar(gt[:], cur[:], tb[:, 1:2], None,
                                                op0=OP.is_gt)
                        nxt = rt.tile([128, 16], F32, tag="lm")
                        nc.vector.select(nxt[:], gt[:], neg[:], cur[:])
                        cur = nxt
                tb3 = rt.tile([128, 2], F32, tag="tb3")
                nc.gpsimd.partition_broadcast(tb3[:], kout[:])
                m16 = rt.tile([16, 128], F32)
                nc.vector.tensor_scalar(m16[:], l16[:], tb3[:16, 1:2], None,
                                        op0=OP.is_gt)
                mi = rt.tile([16, 128], F32)
                nc.vector.tensor_tensor(mi[:], iota1[:], m16[:], op=OP.mult)
                nc.vector.tensor_scalar_add(mi[:], mi[:], -1.0)
                compact = rt.tile([16, 64], F32)
                nfound = rt.tile([1, 1], U32)
                nc.gpsimd.sparse_gather(compact[:], mi[:], num_found=nfound[:])
                nc.sync.dma_start(scr_c[:], compact[:])
                rows16 = rt.tile([128, 1, 64], F32)
                nc.gpsimd.dma_gather(rows16[:],
                                     scr_c[:].rearrange("(r c) -> r c", c=64),
                                     rowsel[:], num_idxs=8, num_idxs_reg=8,
                                     elem_size=64)
                nc.sync.dma_start(scr_m[:], rows16[0:8, 0, :])
                nc.sync.dma_start(idx_d, scr_m[:].rearrange("(f p) -> p f", p=16))
                mc_r = rt.tile([16, 32], F32R)
                nc.sync.dma_start(mc_r[:],
                                  scr_m[:].rearrange("(f p) -> p f", p=16)
                                  .bitcast(F32R))
                idxp = rt_ps.tile([128, 32], F32, tag="idxp")
                nc.tensor.matmul(idxp[:], tile16[:], mc_r[:], start=True, stop=True)
                nc.vector.tensor_copy(idx16[:], idxp[:])

                # vlnT
                for ch in range(NCH):
                    cs = slice(128 * ch, 128 * (ch + 1))
                    for nb in range(4):
                        sl = slice(512 * nb, 512 * (nb + 1))
                        rbp = ps_bc.tile([128, 512], F32, tag="rb")
                        nc.tensor.matmul(rbp[:], vg[:, cs], rstd0[:, sl],
                                         start=True, stop=True)
                        abp = ps_bc.tile([128, 512], F32, tag="ab")
                        nc.tensor.matmul(abp[:], vbg[:, cs], nmr[:, sl],
                                         start=True, stop=True)
                        vt = vlnT[:, ch, sl]
                        nc.vector.tensor_tensor(vt, seqT[:, ch, sl].bitcast(F32),
                                                rbp[:], op=OP.mult)
                        nc.vector.tensor_tensor(vt, vt.bitcast(F32), abp[:],
                                                op=OP.add)

            # ============ A4: q path ============
            pqr = vctx.enter_context(tc.tile_pool(name="qTrp", bufs=1))
            qTr = pqr.tile([128, NCH, NQ], F32R)
            pwb2 = vctx.enter_context(tc.tile_pool(name="wbp", bufs=1))
            w_b = pwb2.tile([128, NQ], F32)
            with tc.tile_pool(name="qgat", bufs=1) as pqg, \
                 tc.tile_pool(name="qw", bufs=8) as pqw, \
                 tc.tile_pool(name="ps_q", bufs=2, space="PSUM") as ps_q, \
                 tc.tile_pool(name="ps_q2", bufs=1, space="PSUM") as ps_q2, \
                 tc.tile_pool(name="ps_tr", bufs=2, space="PSUM") as ps_tr:
                qlnT = pqg.tile([128, NCH, NQ], F32R)
                qseq = pqg.tile([128, 4, D], F32)
                nc.gpsimd.dma_gather(qseq[:], seqn_d, idx16[:], num_idxs=NQ,
                                     num_idxs_reg=NQ, elem_size=D)
                rotq_g = pqg.tile([128, 4, 128], F32)
                nc.gpsimd.dma_gather(rotq_g[:], rotq_d, idx16[:], num_idxs=NQ,
                                     num_idxs_reg=NQ, elem_size=128)
                bst = pqg.tile([128, 4, 2, 6], F32)
                for g in range(4):
                    for hh in range(2):
                        nc.vector.bn_stats(bst[:, g, hh, :],
                                           qseq[:, g, 512 * hh:512 * (hh + 1)])
                mv = pqg.tile([128, 4, 2], F32)
                for g in range(4):
                    nc.vector.bn_aggr(mv[:, g, :], bst[:, g, :, :])
                sd_s = pqg.tile([128, 4], F32)
                eps128 = pqg.tile([128, 1], F32)
                nc.vector.memset(eps128[:], LN_EPS)
                nc.scalar.activation(sd_s[:], mv[:, :, 1], AF.Sqrt, bias=eps128[:])
                rstd_s = pqg.tile([128, 4], F32)
                nc.vector.reciprocal(rstd_s[:], sd_s[:])
                qn = pqg.tile([128, 4, D], F32)
                for g in range(4):
                    nc.vector.tensor_scalar(qn[:, g, :], qseq[:, g, :],
                                            mv[:, g, 0:1], rstd_s[:, g:g + 1],
                                            op0=OP.subtract, op1=OP.mult)
                for g in range(4):
                    for ch in range(NCH):
                        tp = ps_tr.tile([128, 128], F32, tag="tp")
                        nc.tensor.transpose(tp[:], qn[:, g, 128 * ch:128 * (ch + 1)],
                                            ident[:])
                        nc.vector.tensor_scalar(qlnT[:, ch, 128 * g:128 * (g + 1)],
                                                tp[:], qg[:, ch:ch + 1],
                                                qb[:, ch:ch + 1], op0=OP.mult,
                                                op1=OP.add)
                rotqT = pqg.tile([128, NQ], F32)
                for g in range(4):
                    tp = ps_tr.tile([128, 128], F32, tag="tp")
                    nc.tensor.transpose(tp[:], rotq_g[:, g, :], ident[:])
                    nc.vector.tensor_copy(rotqT[:, 128 * g:128 * (g + 1)], tp[:])
                for jch in range(NCH):
                    wq = []
                    for ch in range(NCH):
                        w = pqw.tile([128, 128], F32R, tag="qw")
                        nc.sync.dma_start(w[:], qw_d[128 * ch:128 * (ch + 1),
                                                     128 * jch:128 * (jch + 1)]
                                          .bitcast(F32R))
                        wq.append(w)
                    qp = ps_q.tile([128, NQ], F32, tag="qp")
                    for ch in range(NCH):
                        nc.tensor.matmul(qp[:], wq[ch], qlnT[:, ch, :],
                                         start=(ch == 0), stop=(ch == NCH - 1))
                    nc.vector.tensor_tensor(qTr[:, jch, :], qp[:], rotqT[:],
                                            op=OP.mult)
                nc.sync.dma_start(qspill[:], qlnT[:])
                e16f = pqg.tile([16, S], F32)
                for nb in range(4):
                    sl = slice(512 * nb, 512 * (nb + 1))
                    ep = ps_q2.tile([16, 512], F32, tag="ep")
                    nc.tensor.matmul(ep[:], ones1[:, 0:16], e_row[:, sl],
                                     start=True, stop=True)
                    nc.vector.tensor_copy(e16f[:, sl], ep[:])
                esel = pqg.tile([16, NQ], F32)
                nc.gpsimd.ap_gather(esel[:], e16f[:], idx16[0:16, :], channels=16,
                                    num_elems=S, d=1, num_idxs=NQ)
                w_row = pqg.tile([1, NQ], F32R)
                nc.vector.tensor_scalar(w_row[:], esel[0:1, :], rzr[:], None,
                                        op0=OP.mult)
                wp = ps_q2.tile([128, NQ], F32, tag="wp")
                nc.tensor.matmul(wp[:], ones1[:], w_row[:], start=True, stop=True)
                nc.vector.tensor_copy(w_b[:], wp[:])

            with tc.tile_pool(name="attn", bufs=1) as pattn:
                attn = pattn.tile([128, NJCH, NQ], F32R)
                # ============ B: kT + V + attention ============
                with tc.tile_pool(name="rotkp", bufs=1) as prk, \
                     tc.tile_pool(name="kstr", bufs=2) as pks, \
                     tc.tile_pool(name="v2p", bufs=1) as pv2, \
                     tc.tile_pool(name="kw", bufs=8) as pkw, \
                     tc.tile_pool(name="vw", bufs=8) as pvw, \
                     tc.tile_pool(name="probs", bufs=1) as ppr, \
                     tc.tile_pool(name="rbp", bufs=1) as prb, \
                     tc.tile_pool(name="ps_mm", bufs=1, space="PSUM") as ps_mm, \
                     tc.tile_pool(name="ps_s0", bufs=1, space="PSUM") as ps_s0, \
                     tc.tile_pool(name="ps_s1", bufs=1, space="PSUM") as ps_s1, \
                     tc.tile_pool(name="ps_at", bufs=2, space="PSUM") as ps_at:
                    rotk = prk.tile([128, S], F32)
                    nc.sync.dma_start(rotk[:], rotk_d)
                    v2 = None
                    for jch in range(NJCH):
                        wt = []
                        for ch in range(NCH):
                            w = pkw.tile([128, 128], F32R, tag="kw")
                            nc.sync.dma_start(
                                w[:], kvwk_d[128 * ch:128 * (ch + 1),
                                             128 * jch:128 * (jch + 1)]
                                .bitcast(F32R))
                            wt.append(w)
                        kt = pks.tile([128, S], F32R, tag="kt")
                        for nb in range(4):
                            sl = slice(512 * nb, 512 * (nb + 1))
                            kp = ps_mm.tile([128, 512], F32, tag="mm")
                            for ch in range(NCH):
                                nc.tensor.matmul(kp[:], wt[ch], vlnT[:, ch, sl],
                                                 start=(ch == 0),
                                                 stop=(ch == NCH - 1))
                            nc.vector.tensor_tensor(kt[:, sl], kp[:], rotk[:, sl],
                                                    op=OP.mult)
                        if jch % 2 == 0:
                            wv = []
                            for ch in range(NCH):
                                w = pvw.tile([128, 256], F32R, tag="vw")
                                nc.sync.dma_start(
                                    w[:], kvwv_d[128 * ch:128 * (ch + 1),
                                                 128 * jch:128 * (jch + 2)]
                                    .bitcast(F32R))
                                wv.append(w)
                            v2 = pv2.tile([128, NSCH, 4, VE], F32R, tag="v2")
                            nc.vector.tensor_scalar(
                                v2[:, :, :, HD:],
                                wv[0][:, 0:128].bitcast(F32).rearrange(
                                    "p (a b c) -> p a b c", b=4, c=2),
                                0.0, 1.0, op0=OP.mult, op1=OP.add)
                            for sch in range(NSCH):
                                vp = ps_mm.tile([128, 256], F32, tag="mm")
                                for ch in range(NCH):
                                    nc.tensor.matmul(
                                        vp[:],
                                        vlnT[:, ch, 128 * sch:128 * (sch + 1)],
                                        wv[ch], start=(ch == 0),
                                        stop=(ch == NCH - 1))
                                nc.vector.tensor_copy(
                                    v2[:, sch, :, 0:HD],
                                    vp[:].rearrange("p (h v) -> p h v", v=HD))
                        hb = 2 * (jch % 2)
                        at0 = ps_at.tile([VE, NQ], F32, tag="at")
                        at1 = ps_at.tile([VE, NQ], F32, tag="at")
                        for grp in range(8):
                            sc0 = ps_s0.tile([128, 2, NQ], F32, tag="sc0")
                            sc1 = ps_s1.tile([128, 2, NQ], F32, tag="sc1")
                            for k in range(2):
                                sch = 2 * grp + k
                                ksl = kt[:, 128 * sch:128 * (sch + 1)]
                                nc.tensor.matmul(sc0[:, k, :], ksl[0:64, :],
                                                 qTr[0:64, jch, :], start=True,
                                                 stop=True, tile_position=(0, 0))
                                nc.tensor.matmul(sc1[:, k, :], ksl[64:128, :],
                                                 qTr[64:128, jch, :], start=True,
                                                 stop=True, tile_position=(64, 0))
                            pr0 = ppr.tile([128, 2, NQ], F32R, tag="pr0")
                            pr1 = ppr.tile([128, 2, NQ], F32R, tag="pr1")
                            nc.scalar.activation(pr0[:], sc0[:], AF.Exp)
                            nc.scalar.activation(pr1[:], sc1[:], AF.Exp)
                            for k in range(2):
                                sch = 2 * grp + k
                                nc.tensor.matmul(at0[:], v2[:, sch, hb, :],
                                                 pr0[:, k, :], start=(sch == 0),
                                                 stop=(sch == NSCH - 1))
                                nc.tensor.matmul(at1[:], v2[:, sch, hb + 1, :],
                                                 pr1[:, k, :], start=(sch == 0),
                                                 stop=(sch == NSCH - 1))
                        for hh, at in ((0, at0), (1, at1)):
                            rr_t = prb.tile([VE, NQ], F32, tag="rrt")
                            nc.vector.reciprocal(rr_t[:, :], at[:, :])
                            rr = prb.tile([1, NQ], F32R, tag="rr")
                            nc.sync.dma_start(rr[:],
                                              rr_t[HD:HD + 1, :].bitcast(F32R))
                            rp = ps_s0.tile([64, NQ], F32, tag="rp")
                            nc.tensor.matmul(rp[:], ones1[:, 0:64], rr[:],
                                             start=True, stop=True)
                            rsb = prb.tile([64, NQ], F32, tag="rsb")
                            nc.vector.tensor_copy(rsb[:], rp[:])
                            if hh == 0:
                                nc.vector.tensor_tensor(attn[0:64, jch, :],
                                                        at[0:HD, :], rsb[:],
                                                        op=OP.mult)
                            else:
                                tmp = prb.tile([64, NQ], F32R, tag="tmp")
                                nc.vector.tensor_tensor(tmp[:], at[0:HD, :],
                                                        rsb[:], op=OP.mult)
                                nc.sync.dma_start(attn[64:128, jch, :], tmp[:])

                # ============ C: FFN + out-proj + update ============
                with tc.tile_pool(name="sTp", bufs=1) as psT, \
                     tc.tile_pool(name="qstr", bufs=1) as pqs, \
                     tc.tile_pool(name="fw", bufs=8) as pfw, \
                     tc.tile_pool(name="silu", bufs=3) as psl, \
                     tc.tile_pool(name="updp", bufs=2) as pup, \
                     tc.tile_pool(name="ps_c", bufs=1, space="PSUM") as ps_c:
                    sT = psT.tile([128, NCH, NQ], F32R)
                    qstr = pqs.tile([128, NCH, NQ], F32R)
                    nc.sync.dma_start(qstr[:], qspill[:])
                    for j2 in range(NCH):
                        wx, wg2 = [], []
                        for ch in range(NCH):
                            w = pfw.tile([128, 128], F32R, tag="f1x")
                            nc.sync.dma_start(
                                w[:], f1w_d[128 * ch:128 * (ch + 1),
                                            128 * j2:128 * (j2 + 1)]
                                .bitcast(F32R))
                            wx.append(w)
                            w2 = pfw.tile([128, 128], F32R, tag="f1g")
                            nc.sync.dma_start(
                                w2[:], f1w_d[128 * ch:128 * (ch + 1),
                                             D + 128 * j2:D + 128 * (j2 + 1)]
                                .bitcast(F32R))
                            wg2.append(w2)
                        xp = ps_c.tile([128, NQ], F32, tag="xp")
                        for ch in range(NCH):
                            nc.tensor.matmul(xp[:], wx[ch], qstr[:, ch, :],
                                             start=(ch == 0), stop=(ch == NCH - 1))
                        gp = ps_c.tile([128, NQ], F32, tag="gp")
                        for ch in range(NCH):
                            nc.tensor.matmul(gp[:], wg2[ch], qstr[:, ch, :],
                                             start=(ch == 0), stop=(ch == NCH - 1))
                        x1 = psl.tile([128, NQ], F32, tag="x1")
                        nc.vector.tensor_scalar(x1[:], xp[:], f1b[:, j2:j2 + 1],
                                                None, op0=OP.add)
                        # silu(g) = g * sigmoid(g); CoreSim lacks Silu
                        gb = psl.tile([128, NQ], F32, tag="gb")
                        nc.vector.tensor_scalar(gb[:], gp[:],
                                                f1b[:, 8 + j2:9 + j2],
                                                None, op0=OP.add)
                        sl_t = psl.tile([128, NQ], F32, tag="slt")
                        nc.scalar.activation(sl_t[:], gp[:], AF.Sigmoid,
                                             bias=f1b[:, 8 + j2:9 + j2])
                        nc.vector.tensor_tensor(sl_t[:], sl_t[:], gb[:],
                                                op=OP.mult)
                        nc.vector.tensor_tensor(sT[:, j2, :], sl_t[:], x1[:],
                                                op=OP.mult)

                    for ich in range(NCH):
                        wo, wf = [], []
                        for ch in range(NCH):
                            w = pfw.tile([128, 128], F32R, tag="ow")
                            nc.sync.dma_start(
                                w[:], ow_d[128 * ch:128 * (ch + 1),
                                           128 * ich:128 * (ich + 1)]
                                .bitcast(F32R))
                            wo.append(w)
                            w2 = pfw.tile([128, 128], F32R, tag="f2")
                            nc.sync.dma_start(
                                w2[:], f2w_d[128 * ch:128 * (ch + 1),
                                             128 * ich:128 * (ich + 1)]
                                .bitcast(F32R))
                            wf.append(w2)
                        op_ = ps_c.tile([128, NQ], F32, tag="op")
                        for ch in range(NCH):
                            nc.tensor.matmul(op_[:], wo[ch], attn[:, ch, :],
                                             start=(ch == 0), stop=(ch == NCH - 1))
                        fp = ps_c.tile([128, NQ], F32, tag="fp")
                        for ch in range(NCH):
                            nc.tensor.matmul(fp[:], wf[ch], sT[:, ch, :],
                                             start=(ch == 0), stop=(ch == NCH - 1))
                        fs = pup.tile([128, NQ], F32, tag="fs")
                        nc.vector.tensor_scalar(fs[:], fp[:], f2b[:, ich:ich + 1],
                                                None, op0=OP.add)
                        ut = pup.tile([128, NQ], F32, tag="ut")
                        nc.vector.tensor_tensor(ut[:], op_[:], fs[:], op=OP.add)
                        nc.vector.tensor_tensor(ut[:], ut[:], w_b[:], op=OP.mult)
                        nc.sync.dma_start(upd_d[:, ich, :], ut[:])


def _rope_table():
    freqs = np.exp(np.linspace(0.0, -1.0, HD // 2) * np.log(ROPE_BASE))
    pos = np.arange(S, dtype=np.float64)
    ang = pos[:, None] * freqs[None, :].astype(np.float64)
    rot = np.concatenate([np.sin(ang), np.cos(ang)], axis=1)
    return rot.astype(np.float32)


def _make_host_tables():
    rot = _rope_table()                          # (S, 64)
    rotk = np.empty((128, S), np.float32)
    for r in range(128):
        rotk[r] = rot[:, r % HD]
    rotq = np.ascontiguousarray(
        np.concatenate([rot, rot], axis=1) / np.float32(8.0))
    tile16 = np.zeros((16, 128), np.float32)
    for m in range(128):
        tile16[m % 16, m] = 1.0
    iota1 = np.empty((16, 128), np.float32)
    for p in range(16):
        for f in range(128):
            iota1[p, f] = f * 16 + p + 1
    ident = np.eye(128, dtype=np.float32)
    return rotk, rotq, tile16, iota1, ident


def make_in_maps(inputs, n_cores=8):
    seq = np.ascontiguousarray(np.asarray(inputs["seq"], np.float32))
    rotk, rotq, tile16, iota1, ident = _make_host_tables()
    kvT = np.ascontiguousarray(np.asarray(inputs["kv_w"], np.float32).T)
    shared = {
        "statw": np.ascontiguousarray(
            np.stack([np.asarray(inputs["router_w"], np.float32).reshape(D),
                      np.ones(D, np.float32)], axis=1)),
        "kvwk": np.ascontiguousarray(kvT[:, :D]),
        "kvwv": np.ascontiguousarray(kvT[:, D:]),
        "qw": np.ascontiguousarray(np.asarray(inputs["q_w"], np.float32).T),
        "ow": np.ascontiguousarray(np.asarray(inputs["out_w"], np.float32).T),
        "f1w": np.ascontiguousarray(np.asarray(inputs["fc1_w"], np.float32).T),
        "f2w": np.ascontiguousarray(np.asarray(inputs["fc2_w"], np.float32).T),
        "rotk": rotk, "rotq": rotq, "ident": ident,
        "tile16": tile16, "iota1": iota1,
        "vg": np.asarray(inputs["vln_g"], np.float32).reshape(1, D).copy(),
        "vbg": np.ascontiguousarray(
            np.stack([np.asarray(inputs["vln_b"], np.float32),
                      np.asarray(inputs["vln_g"], np.float32)])),
        "qg": np.ascontiguousarray(
            np.asarray(inputs["qln_g"], np.float32).reshape(NCH, 128).T),
        "qb": np.ascontiguousarray(
            np.asarray(inputs["qln_b"], np.float32).reshape(NCH, 128).T),
        "f1b": np.ascontiguousarray(
            np.asarray(inputs["fc1_b"], np.float32).reshape(16, 128).T),
        "f2b": np.ascontiguousarray(
            np.asarray(inputs["fc2_b"], np.float32).reshape(NCH, 128).T),
    }
    in_maps = []
    for c in range(n_cores):
        b, p = c // 2, c % 2
        rowsel = np.empty((128, 1), np.int16)
        for P in range(128):
            rowsel[P, 0] = 8 * p + (P % 16) % 8
        m = dict(shared)
        m["seqT"] = np.ascontiguousarray(seq[b].T)
        m["seqn"] = np.ascontiguousarray(seq[b])
        m["rowsel"] = rowsel
        in_maps.append(m)
    return in_maps


def assemble_output(seq, results):
    out = np.array(seq, np.float32, copy=True)
    for c, r in enumerate(results):
        b = c // 2
        upd = r["updT"].transpose(1, 0, 2).reshape(D, NQ)   # [i, q]
        idxw = np.asarray(r["idxsel"])
        idx = np.empty(NQ, np.int64)
        for j in range(NQ):
            idx[j] = int(idxw[j % 16, j // 16])
        out[b, idx, :] += upd.T
    return out


_PROGRAM = None


def kernel(**inputs):
    global _PROGRAM
    seq = np.asarray(inputs["seq"], np.float32)
    if _PROGRAM is None:
        _PROGRAM = build_program(8)
    in_maps = make_in_maps(inputs, 8)
    res = run_bass_kernel_spmd(_PROGRAM, in_maps, list(range(8)))
    return assemble_output(seq, res.results)

